# revision 8
# baseline (speedup 1.0000x reference)
"""Distributed GQA attention kernel for 8 TRN2 NeuronCores.

Strategy: tensor-parallel over heads, zero collectives.
Each core d holds 4 query heads + 1 kv head (GQA group d). It computes
q/k/v projections (transposed layouts), RoPE, causal attention, and a
partial o_proj (its heads' contribution to every output element). The
host sums the 8 partial outputs (the "unshard" step).

v2: software-pipelined emission. The per-token-block phases
(projection -> rope -> attention -> o_proj) are woven across three
token blocks so the PE never waits on the DVE/scalar chains:
attention(tb) segments are interleaved at matmul granularity with
projection chunks of tb+2 and o_proj groups of tb-1. Softmax exp is
batched over key-block pairs on the scalar engine (which also does
the o_proj PSUM evacuation; exp/copy share an activation table), and
the initial weight DMAs are split into pieces emitted just ahead of
first use so the first projection starts within ~2us.

Precision: bf16 matmuls with f32 PSUM accumulation; softmax in f32
without max-subtraction (scores are ~N(0,1); exp cannot overflow).
RoPE uses a head-dim permutation (even indices first) applied to
wq/wk/wo rows on the host, turning the rotate-pair swap into a
contiguous 64-partition shift on device.
"""
import sys

sys.path.insert(0, '/opt/trn_rl_repo')

import numpy as np
import ml_dtypes

B, T, C = 2, 2048, 4096
H, KVH, HD = 32, 8, 128
NCORES = 8
N = B * T            # 4096 tokens (batches concatenated)
HL = H // NCORES     # 4 local q heads
TB = 256             # token block (q-tile)
NTB = N // TB        # 16
KB = 128             # key block
NCH = C // 128       # 32 contraction chunks
SCALE = float(1.0 / np.sqrt(HD))
PERM = np.concatenate([np.arange(0, 128, 2), np.arange(1, 128, 2)])

BF16 = ml_dtypes.bfloat16


def _build(dbg=False):
    import concourse.mybir as mybir
    import concourse.tile as tile
    from concourse import bacc

    dt = mybir.dt
    nc = bacc.Bacc("TRN2", target_bir_lowering=False, debug=False)

    xT_d = nc.declare_dram_parameter("xT", [C, N], dt.bfloat16, isOutput=False)
    wqT_d = nc.declare_dram_parameter("wqT", [C, HL * HD], dt.bfloat16, isOutput=False)
    wkT_d = nc.declare_dram_parameter("wkT", [C, HD], dt.bfloat16, isOutput=False)
    wvT_d = nc.declare_dram_parameter("wvT", [C, HD], dt.bfloat16, isOutput=False)
    woT_d = nc.declare_dram_parameter("woT", [HL * HD, C], dt.bfloat16, isOutput=False)
    cosb_d = nc.declare_dram_parameter("cosb", [128, N], dt.float32, isOutput=False)
    sinb_d = nc.declare_dram_parameter("sinb", [128, N], dt.float32, isOutput=False)
    mask_d = nc.declare_dram_parameter("mask", [128, 2 * TB], dt.bfloat16, isOutput=False)
    out_d = nc.declare_dram_parameter("out", [N, C], dt.bfloat16, isOutput=True)

    with tile.TileContext(nc) as tc:
        with (
            tc.tile_pool(name="wts", bufs=1) as wts,
            tc.tile_pool(name="cache", bufs=1) as cache,
            tc.tile_pool(name="xin", bufs=12) as xin,
            tc.tile_pool(name="qk", bufs=12) as qkp,
            tc.tile_pool(name="rope", bufs=9) as ropep,
            tc.tile_pool(name="pt", bufs=6) as ptp,
            tc.tile_pool(name="att", bufs=8) as attp,
            tc.tile_pool(name="dn", bufs=2) as dnp,
            tc.tile_pool(name="oev", bufs=4) as oevp,
            tc.tile_pool(name="acc", bufs=3, space="PSUM") as accp,
            tc.tile_pool(name="sps", bufs=2, space="PSUM") as spsp,
            tc.tile_pool(name="atd", bufs=2, space="PSUM") as atdp,
            tc.tile_pool(name="ops", bufs=1, space="PSUM") as opsp,
        ):
            # ---------------- resident weights / constants ----------------
            wq_s = wts.tile([128, NCH * HL * 128], dt.bfloat16)   # (c,h) -> col (c*HL+h)*128
            wk_s = wts.tile([128, NCH * 128], dt.bfloat16)
            wv_s = wts.tile([128, NCH * 128], dt.bfloat16)
            wo_s = wts.tile([128, HL * C], dt.bfloat16)           # (h,ct) -> col h*C+ct*512
            cos_s = wts.tile([128, N], dt.float32)
            sin_s = wts.tile([128, N], dt.float32)
            mask_s = wts.tile([128, 2 * TB], dt.bfloat16)
            ones_s = wts.tile([128, 128], dt.bfloat16)

            nc.any.memset(ones_s[:, :], 1.0)

            wq_v = wq_s[:, :].rearrange("p (c m) -> p c m", c=NCH)
            wqT_v = wqT_d[:, :].rearrange("(c p) m -> p c m", p=128)
            wk_v = wk_s[:, :].rearrange("p (c m) -> p c m", c=NCH)
            wkT_v = wkT_d[:, :].rearrange("(c p) m -> p c m", p=128)
            wv_v = wv_s[:, :].rearrange("p (c m) -> p c m", c=NCH)
            wvT_v = wvT_d[:, :].rearrange("(c p) m -> p c m", p=128)
            wo_v = wo_s[:, :].rearrange("p (h m) -> p h m", h=HL)
            woT_v = woT_d[:, :].rearrange("(h p) m -> p h m", p=128)

            def wq_piece(i):   # i in 0..7, chunks 4i..4i+3
                cs = slice(i * 4, (i + 1) * 4)
                nc.sync.dma_start(wq_v[:, cs], wqT_v[:, cs])

            def wk_piece(i):   # i in 0..3, chunks 8i..8i+7
                cs = slice(i * 8, (i + 1) * 8)
                nc.sync.dma_start(wk_v[:, cs], wkT_v[:, cs])

            def wv_piece(i):
                cs = slice(i * 8, (i + 1) * 8)
                nc.sync.dma_start(wv_v[:, cs], wvT_v[:, cs])

            def cs_piece(i):   # i in 0..3, tokens 1024i..
                ns = slice(i * (N // 4), (i + 1) * (N // 4))
                nc.sync.dma_start(cos_s[:, ns], cosb_d[:, ns])
                nc.sync.dma_start(sin_s[:, ns], sinb_d[:, ns])

            def wo_piece(i):   # i in 0..3 -> head i
                nc.sync.dma_start(wo_v[:, i], woT_v[:, i])

            kcache = cache.tile([128, N], dt.bfloat16)   # [hd, tok]
            vcache = cache.tile([128, N], dt.bfloat16)   # [tok%128, blk*128+hd]

            # ------------- pipeline state, keyed by token block -------------
            acc_t = {}   # tb -> (t0, t1, t2) PSUM proj tiles
            qsb = {}     # tb -> [4 q tiles, rope'd, bf16]
            atds = {}    # (tb, h) -> atd PSUM tile (at | den)
            ahs = {}     # tb -> [4 normalized attention tiles, bf16]

            def emit_proj_chunk(tb, c):
                nsl = slice(tb * TB, (tb + 1) * TB)
                if c == 0:
                    t0 = accp.tile([128, 512], dt.float32, tag="acc")  # q0|q1
                    t1 = accp.tile([128, 512], dt.float32, tag="acc")  # q2|q3
                    t2 = accp.tile([128, 512], dt.float32, tag="acc")  # k|v0|v1
                    acc_t[tb] = (t0, t1, t2)
                t0, t1, t2 = acc_t[tb]
                qps = [t0[:, 0:256], t0[:, 256:512], t1[:, 0:256], t1[:, 256:512]]
                kps = t2[:, 0:256]
                vps = [t2[:, 256:384], t2[:, 384:512]]
                xc = xin.tile([128, TB], dt.bfloat16, tag="xc")
                nc.sync.dma_start(xc[:, :], xT_d[c * 128:(c + 1) * 128, nsl])
                st = c == 0
                sp = c == NCH - 1
                # start=True clears has_written for the WHOLE bank, so only
                # the first matmul touching each bank may set it; sibling
                # slices overwrite via cleared has_written on their first
                # write (PE executes in program order).
                for h in range(HL):
                    nc.tensor.matmul(
                        qps[h], wq_s[:, (c * HL + h) * 128:(c * HL + h + 1) * 128],
                        xc[:, :], start=st and h % 2 == 0, stop=sp)
                nc.tensor.matmul(
                    kps, wk_s[:, c * 128:(c + 1) * 128], xc[:, :],
                    start=st, stop=sp)
                for ti in range(2):
                    nc.tensor.matmul(
                        vps[ti], xc[:, ti * 128:(ti + 1) * 128],
                        wv_s[:, c * 128:(c + 1) * 128], start=False, stop=sp)

            def emit_rope(tb):
                # RoPE + evacuate q (4 heads) and k; copy v to cache.
                nsl = slice(tb * TB, (tb + 1) * TB)
                t0, t1, t2 = acc_t.pop(tb)
                qps = [t0[:, 0:256], t0[:, 256:512], t1[:, 0:256], t1[:, 256:512]]
                kps = t2[:, 0:256]
                vps = [t2[:, 256:384], t2[:, 384:512]]
                q_list = []
                for h in range(HL + 1):  # h==HL is k
                    src = kps if h == HL else qps[h]
                    m1 = ropep.tile([128, TB], dt.float32, tag="m1")
                    nc.vector.tensor_mul(m1[:, :], src, cos_s[:, nsl])
                    u = ropep.tile([128, TB], dt.float32, tag="u")
                    nc.vector.tensor_mul(u[:, :], src, sin_s[:, nsl])
                    sw = ropep.tile([128, TB], dt.float32, tag="sw")
                    nc.sync.dma_start(sw[0:64, :], u[64:128, :])
                    nc.sync.dma_start(sw[64:128, :], u[0:64, :])
                    if h == HL:
                        nc.vector.tensor_add(kcache[:, nsl], m1[:, :], sw[:, :])
                    else:
                        qh = qkp.tile([128, TB], dt.bfloat16, tag="qh")
                        nc.vector.tensor_add(qh[:, :], m1[:, :], sw[:, :])
                        q_list.append(qh)
                qsb[tb] = q_list
                for ti in range(2):
                    kbg = tb * 2 + ti
                    nc.vector.tensor_copy(
                        vcache[:, kbg * 128:(kbg + 1) * 128], vps[ti])

            def emit_attn_segment(tb, hp, jp, npair):
                # head pair hp (heads 2hp, 2hp+1), key-block pair jp.
                b = tb // 8
                diag = jp == npair - 1
                heads = (2 * hp, 2 * hp + 1)
                pts = {}
                for h in heads:
                    sT = spsp.tile([128, 512], dt.float32, tag="sT")
                    for j in range(2):
                        kbg = b * 16 + 2 * jp + j
                        ksl = slice(kbg * 128, (kbg + 1) * 128)
                        nc.tensor.matmul(
                            sT[:, j * 256:(j + 1) * 256], kcache[:, ksl],
                            qsb[tb][h][:, :], start=j == 0, stop=True)
                    pT = ptp.tile([128, 512], dt.bfloat16, tag="pT")
                    nc.scalar.activation(pT[:, :], sT[:, :],
                                         mybir.ActivationFunctionType.Exp,
                                         scale=SCALE)
                    if diag:
                        nc.vector.tensor_mul(pT[:, :], pT[:, :], mask_s[:, :])
                    pts[h] = pT
                yield  # weave point: exp/mask latency is covered here
                for h in heads:
                    if jp == 0:
                        atds[(tb, h)] = atdp.tile([128, 512], dt.float32,
                                                  tag="atd", name="atd")
                    atd = atds[(tb, h)]
                    at = atd[:, 0:256]
                    den = atd[:, 256:512]
                    pT = pts[h]
                    for j in range(2):
                        kbg = b * 16 + 2 * jp + j
                        ksl = slice(kbg * 128, (kbg + 1) * 128)
                        nc.tensor.matmul(at, vcache[:, ksl],
                                         pT[:, j * 256:(j + 1) * 256],
                                         start=jp == 0 and j == 0,
                                         stop=diag and j == 1)
                    for j in range(2):
                        nc.tensor.matmul(den, ones_s[:, :],
                                         pT[:, j * 256:(j + 1) * 256],
                                         start=False, stop=diag and j == 1)

            def emit_norm(tb, h):
                atd = atds.pop((tb, h))
                denb = dnp.tile([128, TB], dt.float32, tag="denb")
                nc.vector.reciprocal(denb[:, :], atd[:, 256:512])
                ah = attp.tile([128, TB], dt.bfloat16, tag="ah")
                nc.vector.tensor_mul(ah[:, :], atd[:, 0:256], denb[:, :])
                ahs.setdefault(tb, []).append(ah)

            def emit_oproj_group(tb, g):
                ti, ct = g % 2, g // 2
                r0 = tb * TB + ti * 128
                ops = opsp.tile([128, 512], dt.float32, tag="ops")
                for h in range(HL):
                    nc.tensor.matmul(
                        ops[:, :],
                        ahs[tb][h][:, ti * 128:(ti + 1) * 128],
                        wo_s[:, h * C + ct * 512:h * C + (ct + 1) * 512],
                        start=h == 0, stop=h == HL - 1)
                oev = oevp.tile([128, 512], dt.bfloat16, tag="oev")
                nc.scalar.copy(oev[:, :], ops[:, :])
                nc.sync.dma_start(
                    out_d[r0:r0 + 128, ct * 512:(ct + 1) * 512], oev[:, :])
                if g == 15:
                    del ahs[tb]

            # ---------------- prologue: proj(0), proj(1) ----------------
            nc.sync.dma_start(mask_s[:, :], mask_d[:, :])
            wq_piece(0); wk_piece(0); wv_piece(0)
            prol0 = {0: [lambda: wq_piece(1)],
                     2: [lambda: wk_piece(1), lambda: wv_piece(1)],
                     4: [lambda: wq_piece(2)],
                     6: [lambda: wk_piece(2), lambda: wv_piece(2)],
                     8: [lambda: wq_piece(3)],
                     10: [lambda: wk_piece(3), lambda: wv_piece(3)],
                     12: [lambda: wq_piece(4)],
                     16: [lambda: wq_piece(5)],
                     20: [lambda: wq_piece(6)],
                     24: [lambda: wq_piece(7)],
                     28: [lambda: cs_piece(0)]}
            for c in range(NCH):
                emit_proj_chunk(0, c)
                for f in prol0.get(c, ()):
                    f()
            emit_rope(0)
            prol1 = {8: [lambda: cs_piece(1)]}
            for c in range(NCH):
                emit_proj_chunk(1, c)
                for f in prol1.get(c, ()):
                    f()
            emit_rope(1)

            # ---------------- steady-state weave ----------------
            for tb in range(NTB + 1):
                # Interleave proj chunks and oproj groups 2:1 so consecutive
                # oproj groups are never adjacent (the single ops PSUM bank
                # serializes on its scalar-copy evacuation otherwise).
                pw = [(emit_proj_chunk, (tb + 2, c)) for c in range(NCH)] \
                    if tb + 2 < NTB else []
                ow = [(emit_oproj_group, (tb - 1, g)) for g in range(16)] \
                    if tb >= 1 else []
                fill = []
                pi = oi = 0
                while pi < len(pw) or oi < len(ow):
                    for _ in range(2):
                        if pi < len(pw):
                            fill.append(pw[pi]); pi += 1
                    if oi < len(ow):
                        fill.append(ow[oi]); oi += 1
                # deferred weight/const DMA pieces, spread into this tb's fill
                deferred = {0: [(wo_piece, (0,)), (wo_piece, (1,)),
                                (wo_piece, (2,)), (wo_piece, (3,))],
                            2: [(cs_piece, (2,))],
                            4: [(cs_piece, (3,))]}.get(tb, [])
                for k, item in enumerate(deferred):
                    fill.insert(2 + 3 * k, item)
                fpos = 0
                bonus = 0

                def fill_to(total_slots, slot):
                    nonlocal fpos
                    tgt = min(len(fill), len(fill) * slot // total_slots + bonus)
                    while fpos < tgt:
                        fn, args = fill[fpos]
                        fn(*args)
                        fpos += 1

                if tb < NTB:
                    npair = (tb % 8) + 1
                    nslot = 2 * npair * 2  # 2 head pairs x npair x 2 weave points
                    slot = 0
                    for hp in range(2):
                        for jp in range(npair):
                            seg = emit_attn_segment(tb, hp, jp, npair)
                            next(seg)          # scores + exp (+mask)
                            slot += 1
                            fill_to(nslot, slot)
                            for _ in seg:      # av + den
                                pass
                            slot += 1
                            fill_to(nslot, slot)
                        emit_norm(tb, 2 * hp)
                        emit_norm(tb, 2 * hp + 1)
                        # cover the norm chain (DVE reciprocal ~1.7us) before
                        # the next head pair's first av needs the atd slot
                        bonus += 3
                        fill_to(nslot, slot)
                # drain remaining fill
                while fpos < len(fill):
                    fn, args = fill[fpos]
                    fn(*args)
                    fpos += 1
                if tb + 2 < NTB:
                    emit_rope(tb + 2)
    nc.finalize()
    return nc


def _prep_shared(x, freqs_cis):
    xf = np.asarray(x, np.float32).reshape(N, C)
    xT = np.ascontiguousarray(xf.T).astype(BF16)
    fc = np.asarray(freqs_cis, np.float32)
    cos = fc[:, :, 0]
    sin = fc[:, :, 1]
    cosb = np.ascontiguousarray(np.tile(np.concatenate([cos.T, cos.T], 0), (1, B)), dtype=np.float32)
    # pre-swapped sin: device computes u = q*sinb then rotates u by 64
    # partitions, giving swap64(q)*(-sin | +sin) as RoPE needs.
    sinb = np.ascontiguousarray(np.tile(np.concatenate([sin.T, -sin.T], 0), (1, B)), dtype=np.float32)
    j = np.arange(KB)[:, None]
    qq = np.arange(TB)[None, :]
    mask = np.concatenate(
        [(sub * KB + j <= qq).astype(np.float32) for sub in (0, 1)], axis=1
    ).astype(BF16)
    return xT, cosb, sinb, mask


def _prep_core(d, wq_p, wk_p, wv_f, wo_f):
    qsl = slice(d * HL * HD, (d + 1) * HL * HD)
    ksl = slice(d * HD, (d + 1) * HD)
    wqT = np.ascontiguousarray(wq_p[qsl].T).astype(BF16)
    wkT = np.ascontiguousarray(wk_p[ksl].T).astype(BF16)
    wvT = np.ascontiguousarray(wv_f[ksl].T).astype(BF16)
    woT = np.ascontiguousarray(wo_f[:, qsl].T).astype(BF16)
    return wqT, wkT, wvT, woT


_NC_CACHE = []


def kernel(x, freqs_cis, wq, wk, wv, wo):
    from concourse import bass_utils

    if not _NC_CACHE:
        _NC_CACHE.append(_build())
    nc = _NC_CACHE[0]

    xT, cosb, sinb, mask = _prep_shared(x, freqs_cis)
    wq_p = np.asarray(wq, np.float32).reshape(H, HD, C)[:, PERM, :].reshape(H * HD, C)
    wk_p = np.asarray(wk, np.float32).reshape(KVH, HD, C)[:, PERM, :].reshape(KVH * HD, C)
    wv_f = np.asarray(wv, np.float32)
    wo_f = np.asarray(wo, np.float32)

    in_maps = []
    for d in range(NCORES):
        wqT, wkT, wvT, woT = _prep_core(d, wq_p, wk_p, wv_f, wo_f)
        in_maps.append({
            "xT": xT, "wqT": wqT, "wkT": wkT, "wvT": wvT, "woT": woT,
            "cosb": cosb, "sinb": sinb, "mask": mask,
        })
    res = bass_utils.run_bass_kernel_spmd(nc, in_maps, core_ids=list(range(NCORES)))
    acc = np.zeros((N, C), np.float32)
    for r in res.results:
        acc += np.asarray(r["out"], np.float32)
    return acc.reshape(B, T, C)
